# revision 1
# baseline (speedup 1.0000x reference)
"""PoPE attention Trainium2 kernel, 8-core tensor-parallel (2 heads/core).

Self-contained: hardcodes shapes B=1, S=2048, HID=2048, NH=16, HD=128.

Math (per reference):
  q/k/v = X @ w{q,k,v}.T, split into 16 heads of dim 128
  mu_{q,k} = softplus(q/k)
  q_polar = mu_q * (cos/sin)(pos*invfreq);  k uses angles + clipped bias
  scores  = (q_re.k_re + q_im.k_im)/sqrt(128) + causal_mask
  out     = softmax(scores) @ v;  final = out @ wo.T

Sharding: heads 2c,2c+1 on core c (wq/wk/wv column-sharded). The output
projection is COLUMN-sharded on wo: each core multiplies only its own
heads' attention outputs into a full-height [HID, 512] fp16 partial per
512-wide t-chunk, and a per-chunk ReduceScatter(add) sums the partials
and leaves core c with its 256-row slice of the final (transposed)
output, written directly to per-chunk external outputs. Every oproj
matmul needs only LOCAL data, so all compute overlaps the collectives.

Device layout is feature-major ("transposed"): activations live as
[d, s] so every matmul contracts along partitions with zero on-device
transposes. Scores are computed as E[s', t] so softmax's sum reduction
is a ones-vector matmul and E feeds the A@V matmul directly. K's
rotation tables cos/sin(pos*invfreq + bias_h) are folded on the host.
"""

import math
import sys
import types

import numpy as np
import ml_dtypes

import concourse.bass as bass
import concourse.mybir as mybir
import concourse.tile as tile
from concourse.bass_utils import run_bass_kernel_spmd

# ---------------------------------------------------------------- constants
B, S, HID = 1, 2048, 2048
NH, HD = 16, 128
BASE = 10000.0
N_CORES = 8
HPC = NH // N_CORES          # heads per core = 2
DPC = HPC * HD               # head dims per core = 256
P = 128                      # partitions
KO = HID // P                # 16 k-subtiles
NCH = S // 512               # 4 free-dim chunks of 512
SQ = S // P                  # 16 s'-tiles of 128
BF16 = mybir.dt.bfloat16
F16 = mybir.dt.float16
F32 = mybir.dt.float32
AF = mybir.ActivationFunctionType
ALU = mybir.AluOpType
ISQ = 1.0 / math.sqrt(HD)
NEG = -1.0e9


def _install_ntff_hook():
    """Bare agent image lacks antenv.axon_hooks; synthesize it from the boot
    module's ctypes NTFF hook so run_bass_kernel_spmd(trace=True) works."""
    if "antenv.axon_hooks" in sys.modules:
        return
    try:
        from trn_agent_boot.trn_boot import _ntff_profile_via_ctypes
        hook = _ntff_profile_via_ctypes("/opt/axon/libaxon_pjrt.so")
    except Exception:
        hook = None
    mod = types.ModuleType("antenv.axon_hooks")
    mod.get_axon_ntff_profile_hook = lambda: hook
    mod.set_axon_ntff_profile_hook = lambda h: None
    sys.modules["antenv.axon_hooks"] = mod


_install_ntff_hook()

_TPB_ENGINES = (
    mybir.EngineType.PE,
    mybir.EngineType.Activation,
    mybir.EngineType.DVE,
    mybir.EngineType.Pool,
    mybir.EngineType.SP,
)


class SplitDrainTileContext(tile.TileContext):
    """This walrus build allows at most ONE sem wait per TPB instruction.
    Legalize: move extra waits onto single-wait NOPs emitted just before the
    instruction on the same engine, and split the tail drain the same way."""

    def _split_multiwait(self, insts):
        out = []
        for inst in insts:
            si = getattr(inst, "sync_info", None)
            if (
                si is not None
                and si.on_wait
                and len(si.on_wait) > 1
                and inst.engine in _TPB_ENGINES
            ):
                waits = list(si.on_wait)
                for w in waits[:-1]:
                    out.append(
                        mybir.InstNoOp(
                            name=self.nc.get_next_instruction_name(),
                            sync_info=mybir.SyncInfo(on_wait=[w], on_update=[]),
                            bass_nofuse=True,
                            engine=inst.engine,
                        )
                    )
                si.on_wait = waits[-1:]
            out.append(inst)
        return out

    def _lower_ordered_insts(self, ordered):
        for k in list(ordered.keys()):
            ordered[k] = self._split_multiwait(ordered[k])
        return super()._lower_ordered_insts(ordered)

    def _drain_and_barrier(self, tick_clock, wait_clock):
        from concourse.vector_clock import ScopedClock

        drain_inst = self.nc.sync.drain()
        wait_clock.add_sem_waits(
            drain_inst.ins, ScopedClock({None: tick_clock.global_clock})
        )
        waits = list(drain_inst.ins.sync_info.on_wait or [])
        if len(waits) > 1:
            drain_inst.ins.sync_info.on_wait = waits[:1]
            for w in waits[1:]:
                d2 = self.nc.sync.drain()
                if d2.ins.sync_info is None:
                    d2.ins.sync_info = mybir.SyncInfo(on_wait=[w], on_update=[])
                else:
                    d2.ins.sync_info.on_wait = [w]

        self.nc.all_engine_barrier()
        assert self.sems is not None
        popped = self.nc._tile_sem_poison_stack.pop()
        assert popped is self._sem_poison
        self.nc.clear_and_free_semaphores(list(self.sems.allocated().values()))
        self.nc.all_engine_barrier()


def build_nc():
    nc = bass.Bass("TRN2", target_bir_lowering=False, debug=False,
                   num_devices=N_CORES)

    xt_d = nc.dram_tensor("xt", [HID, S], BF16, kind="ExternalInput").ap()
    wq_d = nc.dram_tensor("wq", [HID, DPC], BF16, kind="ExternalInput").ap()
    wk_d = nc.dram_tensor("wk", [HID, DPC], BF16, kind="ExternalInput").ap()
    wv_d = nc.dram_tensor("wv", [HID, DPC], BF16, kind="ExternalInput").ap()
    # wo.T rows for this core's head dims: [DPC (o_local), HID (h_out)]
    wor_d = nc.dram_tensor("wor", [DPC, HID], BF16, kind="ExternalInput").ap()
    cos_d = nc.dram_tensor("cosT", [P, S], BF16, kind="ExternalInput").ap()
    sin_d = nc.dram_tensor("sinT", [P, S], BF16, kind="ExternalInput").ap()
    # per-head K rotation tables cos/sin(freqs + bias_h): [HPC*HD, S]
    ck_d = nc.dram_tensor("ckT", [DPC, S], BF16, kind="ExternalInput").ap()
    sk_d = nc.dram_tensor("skT", [DPC, S], BF16, kind="ExternalInput").ap()
    tri_d = nc.dram_tensor("tri", [P, P], F32, kind="ExternalInput").ap()
    out_d = [nc.dram_tensor(f"out{j}", [DPC, 512], F16,
                            kind="ExternalOutput").ap()
             for j in range(NCH)]

    with SplitDrainTileContext(nc) as tc:
        with tc.tile_pool(name="big", bufs=1) as big, \
             tc.tile_pool(name="wts", bufs=1) as wts, \
             tc.tile_pool(name="tabs", bufs=1) as tabs, \
             tc.tile_pool(name="qk", bufs=2) as qkp, \
             tc.tile_pool(name="mu", bufs=3) as mup, \
             tc.tile_pool(name="ep", bufs=5) as ep, \
             tc.tile_pool(name="sm", bufs=2) as smp, \
             tc.tile_pool(name="ob", bufs=4) as obp, \
             tc.tile_pool(name="pf", bufs=16) as pfp, \
             tc.tile_pool(name="ps", bufs=1, space="PSUM") as psp, \
             tc.tile_pool(name="dram", bufs=1, space="DRAM") as dram:

            # ---------------- loads -----------------------------------
            # xt arrives in four 512-wide column pieces (all ko per piece)
            # so projections for s-chunk n can start after piece n lands.
            # Order: everything piece 0 needs first.
            wq_sb = wts.tile([P, KO, DPC], BF16, name="wq_sb")
            nc.sync.dma_start(wq_sb[:], wq_d.rearrange("(ko p) o -> p ko o", p=P))

            xt_sb = big.tile([P, KO, S], BF16, tag="big", name="xt_sb")
            xr = xt_d.rearrange("(ko p) s -> p ko s", p=P)
            def load_piece(n):
                ch = slice(512 * n, 512 * (n + 1))
                for ko in range(KO):
                    nc.sync.dma_start(xt_sb[:, ko, ch], xr[:, ko, ch])

            load_piece(0)

            wk_sb = wts.tile([P, KO, DPC], BF16, name="wk_sb")
            nc.sync.dma_start(wk_sb[:], wk_d.rearrange("(ko p) o -> p ko o", p=P))
            cos_sb = tabs.tile([P, S], BF16, name="cos_sb")
            nc.sync.dma_start(cos_sb[:], cos_d[:])
            sin_sb = tabs.tile([P, S], BF16, name="sin_sb")
            nc.sync.dma_start(sin_sb[:], sin_d[:])
            ck_sb = tabs.tile([P, HPC, S], BF16, name="ck_sb")
            nc.sync.dma_start(ck_sb[:], ck_d.rearrange("(h p) s -> p h s", p=P))
            sk_sb = tabs.tile([P, HPC, S], BF16, name="sk_sb")
            nc.sync.dma_start(sk_sb[:], sk_d.rearrange("(h p) s -> p h s", p=P))
            tri_sb = tabs.tile([P, P], F32, name="tri_sb")
            nc.sync.dma_start(tri_sb[:], tri_d[:])
            wv_sb = wts.tile([P, KO, DPC], BF16, name="wv_sb")
            nc.sync.dma_start(wv_sb[:], wv_d.rearrange("(ko p) o -> p ko o", p=P))

            for n in range(1, NCH):
                load_piece(n)

            # wo rows: needed first at oproj(0); finishes long before.
            wo_sb = wts.tile([P, HPC, HID], BF16, name="wo_sb")
            nc.sync.dma_start(wo_sb[:], wor_d.rearrange("(h p) o -> p h o", p=P))

            ones_k = tabs.tile([P, 1], BF16, name="ones_k")
            nc.gpsimd.memset(ones_k[:], 1.0)
            ones_m = tabs.tile([1, P], BF16, name="ones_m")
            nc.gpsimd.memset(ones_m[:], 1.0)

            # Collective scratch. RS inputs are local DRAM; outputs are the
            # external per-chunk tensors directly (local DRAM, allowed for
            # ReduceScatter). Warm up the ring early with a 32KB gather so
            # the first real collective doesn't pay ring setup.
            wu_in = dram.tile([P, P], BF16, name="wu_in")
            wu_out = dram.tile([N_CORES * P, P], BF16, addr_space="Shared",
                               name="wu_out")
            rs_in = [dram.tile([NH * HD, 512], F16, name=f"rs_in{j}")
                     for j in range(NCH)]
            rs_out = [dram.tile([DPC, 512], F16, name=f"rs_out{j}")
                      for j in range(NCH)]

            nc.sync.dma_start(wu_in[:], cos_sb[:, 0:P])
            nc.gpsimd.collective_compute(
                "AllGather", ALU.bypass,
                replica_groups=[list(range(N_CORES))],
                ins=[wu_in[:]], outs=[wu_out[:]],
            )

            # ---------------- QKV projections (s-chunk major) ---------
            q_re = {}
            q_im = {}
            k_re = {}
            k_im = {}
            for h in range(HPC):
                q_re[h] = qkp.tile([P, S], BF16, tag="q_re", name=f"q_re{h}")
                q_im[h] = qkp.tile([P, S], BF16, tag="q_im", name=f"q_im{h}")
                k_re[h] = qkp.tile([P, S], BF16, tag="k_re", name=f"k_re{h}")
                k_im[h] = qkp.tile([P, S], BF16, tag="k_im", name=f"k_im{h}")

            for n in range(NCH):
                ch = slice(512 * n, 512 * (n + 1))
                for h in range(HPC):
                    hsl = slice(P * h, P * (h + 1))
                    # --- Q
                    pq = psp.tile([P, 512], F32, tag="pp", bufs=2, name="pq")
                    for ko in range(KO):
                        nc.tensor.matmul(pq[:], wq_sb[:, ko, hsl],
                                         xt_sb[:, ko, ch],
                                         start=(ko == 0), stop=(ko == KO - 1))
                    # softplus(x) = ln(exp(x) + 1); Softplus has no ACT table
                    # set in this build, Exp/Ln share one.
                    eq = mup.tile([P, 512], F32, tag="mu", name="eq")
                    nc.scalar.activation(eq[:], pq[:], AF.Exp)
                    mu = mup.tile([P, 512], F32, tag="mu", name="mu_q")
                    nc.scalar.activation(mu[:], eq[:], AF.Ln, bias=1.0)
                    nc.vector.tensor_tensor(q_re[h][:, ch], mu[:],
                                            cos_sb[:, ch], ALU.mult)
                    nc.vector.tensor_tensor(q_im[h][:, ch], mu[:],
                                            sin_sb[:, ch], ALU.mult)
                    # --- K
                    pk = psp.tile([P, 512], F32, tag="pp", bufs=2, name="pk")
                    for ko in range(KO):
                        nc.tensor.matmul(pk[:], wk_sb[:, ko, hsl],
                                         xt_sb[:, ko, ch],
                                         start=(ko == 0), stop=(ko == KO - 1))
                    ek = mup.tile([P, 512], F32, tag="mu", name="ek")
                    nc.scalar.activation(ek[:], pk[:], AF.Exp)
                    muk = mup.tile([P, 512], F32, tag="mu", name="mu_k")
                    nc.scalar.activation(muk[:], ek[:], AF.Ln, bias=1.0)
                    # k rotation tables carry the per-head bias (host-folded)
                    nc.vector.tensor_tensor(k_re[h][:, ch], muk[:],
                                            ck_sb[:, h, ch], ALU.mult)
                    nc.vector.tensor_tensor(k_im[h][:, ch], muk[:],
                                            sk_sb[:, h, ch], ALU.mult)
                # --- V for the four s'-tiles inside this piece
                for i in range(4 * n, 4 * n + 4):
                    ssl = slice(P * i, P * (i + 1))
                    pv = psp.tile([P, DPC], F32, tag="pp", bufs=2, name="pv")
                    for ko in range(KO):
                        nc.tensor.matmul(pv[:], xt_sb[:, ko, ssl],
                                         wv_sb[:, ko, :],
                                         start=(ko == 0), stop=(ko == KO - 1))
                    if n == 0 and i == 0:
                        v_sb = big.tile([P, SQ, DPC], BF16, tag="vsb",
                                        name="v_sb")
                    nc.vector.tensor_copy(out=v_sb[:, i, :], in_=pv[:])

            # ---------------- attention, t-chunk major ----------------
            # The per-(head,chunk) normalize tail (pb/bc/osb) and the
            # per-chunk oproj+ReduceScatter are deferred until the next
            # block's first scores are in flight, so the PE never
            # head-of-line waits on the ACT recip chain.
            deferred = []
            osb = {}

            def make_finalize(rec, pav, h, j):
                def finalize():
                    # broadcast 1/rowsum over partitions via ones matmul
                    # (bf16 moving: 1 cyc/row). Shares the "prs" PSUM tag
                    # with psum1 so it never couples to the QKV pp tag.
                    pb = psp.tile([P, 512], F32, tag="prs", bufs=2,
                                  name="pb")
                    nc.tensor.matmul(pb[:], ones_m[:], rec[:],
                                     start=True, stop=True)
                    bc = smp.tile([P, 512], F32, tag="bc", name="bc")
                    nc.vector.tensor_copy(out=bc[:], in_=pb[:])
                    o = obp.tile([P, 512], BF16, tag="osb", name=f"osb{j}_{h}")
                    nc.vector.tensor_tensor(o[:], pav[:], bc[:], ALU.mult)
                    osb[(j, h)] = o
                return finalize

            def make_oproj(j):
                def oproj():
                    ch = slice(512 * j, 512 * (j + 1))
                    for m in range(KO):
                        msl = slice(P * m, P * (m + 1))
                        po = psp.tile([P, 512], F32, tag="pp", bufs=2,
                                      name="po")
                        nc.tensor.matmul(po[:], wo_sb[:, 0, msl],
                                         osb[(j, 0)][:],
                                         start=True, stop=False)
                        nc.tensor.matmul(po[:], wo_sb[:, 1, msl],
                                         osb[(j, 1)][:],
                                         start=False, stop=True)
                        pfo = pfp.tile([P, 512], F16, tag="pf", name="pfo")
                        # last chunk: split the casts across DVE and the
                        # (by then idle) ACT engine to halve the RS3 data tail
                        if j == NCH - 1 and m % 2 == 0:
                            nc.scalar.activation(pfo[:], po[:], AF.Copy)
                        else:
                            nc.vector.tensor_copy(out=pfo[:], in_=po[:])
                        nc.sync.dma_start(rs_in[j][P * m:P * (m + 1), :],
                                          pfo[:])
                    nc.gpsimd.collective_compute(
                        "ReduceScatter", ALU.add,
                        replica_groups=[list(range(N_CORES))],
                        ins=[rs_in[j][:]], outs=[rs_out[j][:]],
                    )

                return oproj

            for j in range(NCH):
                for h in range(HPC):
                    hsl = slice(P * h, P * (h + 1))
                    nlive = 4 * j + 4
                    pav = psp.tile([P, 512], F32, tag="pav", bufs=2,
                                   name="pav")
                    psum1 = psp.tile([1, 512], F32, tag="prs", bufs=2,
                                     name="psum1")
                    # software-pipelined (depth 2): emit rowsum/AV for
                    # iteration i-2 after iteration i's exp, so the PE never
                    # head-of-line blocks on an exp that isn't done yet.
                    # Diagonal tiles only touch their valid [t0:] columns, so
                    # no zero-fill of e is ever needed.
                    pend = []

                    def emit_rs_av(e_t, i_t, lv_t, nlive=nlive, pav=pav,
                                   psum1=psum1, hsl=hsl):
                        nc.tensor.matmul(psum1[:, lv_t], ones_k[:],
                                         e_t[:, lv_t],
                                         start=(i_t == 0),
                                         stop=(i_t == nlive - 1))
                        nc.tensor.matmul(pav[:, lv_t], v_sb[:, i_t, hsl],
                                         e_t[:, lv_t],
                                         start=(i_t == 0),
                                         stop=(i_t == nlive - 1))

                    for i in range(nlive):
                        r = i - 4 * j
                        t0 = 0 if r < 0 else P * r
                        tvs = slice(512 * j + t0, 512 * (j + 1))
                        lvs = slice(t0, 512)
                        ps = psp.tile([P, 512], F32, tag="ps", bufs=2,
                                      name="ps")
                        ksl = slice(P * i, P * (i + 1))
                        nc.tensor.matmul(ps[:, lvs], k_re[h][:, ksl],
                                         q_re[h][:, tvs],
                                         start=True, stop=False)
                        nc.tensor.matmul(ps[:, lvs], k_im[h][:, ksl],
                                         q_im[h][:, tvs],
                                         start=False, stop=True)
                        if r >= 0:
                            # diagonal 128-col sub-block gets causal mask
                            nc.vector.tensor_tensor(
                                ps[:, t0:t0 + P], ps[:, t0:t0 + P],
                                tri_sb[:], ALU.add)
                        e = ep.tile([P, 512], BF16, tag="e", name="e")
                        nc.scalar.activation(e[:, lvs], ps[:, lvs], AF.Exp,
                                             scale=ISQ)
                        pend.append((e, i, lvs))
                        if len(pend) > 2:
                            emit_rs_av(*pend.pop(0))
                        if i == 1:
                            for fin in deferred:
                                fin()
                            deferred = []
                    for p_ in pend:
                        emit_rs_av(*p_)
                    # rec = 1/rowsum via exp(-ln(x)): same ACT table set as
                    # the attention exps. Emitted now so psum1 frees early;
                    # the PE-side tail is deferred. bf16 rec keeps the pb
                    # broadcast matmul at 1 cycle/row.
                    lnt = smp.tile([1, 512], F32, tag="lnt", name="lnt")
                    nc.scalar.activation(lnt[:], psum1[:], AF.Ln)
                    rec = smp.tile([1, 512], BF16, tag="rec", name="rec")
                    nc.scalar.activation(rec[:], lnt[:], AF.Exp, scale=-1.0)

                    last = (j == NCH - 1 and h == 1)
                    if last:
                        make_finalize(rec, pav, h, j)()
                        make_oproj(j)()
                    else:
                        deferred.append(make_finalize(rec, pav, h, j))
                        if h == 1:
                            deferred.append(make_oproj(j))
            for fin in deferred:
                fin()
            # RS-output drains on the scalar DGE, emitted dead last: nothing
            # queues behind them, so their waits on RS completions can never
            # stall the compute pipeline or the collective chain.
            for j in range(NCH):
                nc.scalar.dma_start(out_d[j][:], rs_out[j][:])

    return nc


_NC_CACHE = None
_LAST_IN_MAPS = None


def _get_nc():
    global _NC_CACHE
    if _NC_CACHE is None:
        _NC_CACHE = build_nc()
    return _NC_CACHE


def kernel(hidden_states, wq, wk, wv, wo, learned_bias, attention_mask):
    bf16 = ml_dtypes.bfloat16
    x = np.asarray(hidden_states, dtype=np.float32).reshape(S, HID)
    xt = np.ascontiguousarray(x.T).astype(bf16)

    wqT = np.asarray(wq, dtype=np.float32).T.astype(bf16)   # [HID, out]
    wkT = np.asarray(wk, dtype=np.float32).T.astype(bf16)
    wvT = np.asarray(wv, dtype=np.float32).T.astype(bf16)
    woT = np.asarray(wo, dtype=np.float32).T                # [o, h_out]

    inv_freq = 1.0 / (BASE ** (np.arange(HD, dtype=np.float32) / HD))
    pos = np.arange(S, dtype=np.float32)
    freqs = pos[:, None] * inv_freq[None, :]                # [S, HD]
    cosT = np.ascontiguousarray(np.cos(freqs).T).astype(bf16)  # [HD, S]
    sinT = np.ascontiguousarray(np.sin(freqs).T).astype(bf16)

    bias = np.clip(np.asarray(learned_bias, dtype=np.float32),
                   -2.0 * math.pi, 0.0).reshape(NH, HD)     # [NH, HD]
    kang = freqs[None, :, :] + bias[:, None, :]             # [NH, S, HD]
    ckT = np.cos(kang).transpose(0, 2, 1)                   # [NH, HD, S]
    skT = np.sin(kang).transpose(0, 2, 1)

    tri = np.where(np.arange(P)[:, None] > np.arange(P)[None, :],
                   np.float32(NEG), np.float32(0.0)).astype(np.float32)

    in_maps = []
    for c in range(N_CORES):
        osl = slice(DPC * c, DPC * (c + 1))
        heads = slice(HPC * c, HPC * (c + 1))
        in_maps.append({
            "xt": xt,
            "wq": np.ascontiguousarray(wqT[:, osl]),
            "wk": np.ascontiguousarray(wkT[:, osl]),
            "wv": np.ascontiguousarray(wvT[:, osl]),
            "wor": np.ascontiguousarray(woT[osl, :]).astype(bf16),
            "cosT": cosT,
            "sinT": sinT,
            "ckT": np.ascontiguousarray(
                ckT[heads].reshape(DPC, S)).astype(bf16),
            "skT": np.ascontiguousarray(
                skT[heads].reshape(DPC, S)).astype(bf16),
            "tri": tri,
        })

    global _LAST_IN_MAPS
    _LAST_IN_MAPS = in_maps
    nc = _get_nc()
    res = run_bass_kernel_spmd(nc, in_maps, list(range(N_CORES)))
    finalT = np.concatenate(
        [np.concatenate([res.results[c][f"out{j}"] for j in range(NCH)],
                        axis=1)
         for c in range(N_CORES)], axis=0)                   # [HID, S]
    return np.ascontiguousarray(finalT.T)[None].astype(np.float32)

